# revision 1
# baseline (speedup 1.0000x reference)
"""Trainium2 Bass kernel for nn_Aligner (head-summed sparse attention).

Math (per batch b):
  Q = hs @ Wq + bq            [LQ, 384]
  K = x  @ Wk + bk            [LK, 384]
  V = x  @ Wv + bv            [LK, 384]
  S = Q @ K^T / sqrt(192)     (head-sum of per-head scores == full-width dot)
  P = softmax_k(S + (m-1)*inf)
  out = P @ V                 [LQ, 384]

Key restructurings (all exact up to fp):
- Scores are O(1) here, so softmax runs without max subtraction:
  P = exp(S*scale)*m, normalized by the row sum at the end.
- S^T [k, q] layout: probability tiles come out pre-transposed for the
  P^T-stationary PX matmul.
- K-associativity: S^T = X @ (Wk @ Q'^T). R = Wk@Q'^T is a tiny [384, LQ]
  matmul, eliminating the K projection. The bk bias adds a per-q constant
  to S which softmax normalization cancels exactly — dropped.
- V-associativity: out = (P@[X|1]) @ Wv + bv. PX accumulates against the
  raw x rows; the row sum falls out of the ones column; the V projection
  collapses into one final [LQ,384]@[384,384] matmul.
- The additive mask is injected into the S^T PSUM accumulation via
  fp8e5 DoubleRow matmuls of q-subtile-paired mask chunks (stationary)
  against a block-diag(BIG), cancelled by a -BIG*scale bias inside the
  exp; the fp8 rounding of BIG is a uniform per-column factor that the
  normalization cancels exactly ({0,1} mask values are exact in fp8e5).

- The S^T contraction runs 2/3 in fp8e4m3 DoubleRow (x^T features 0:256
  paired with R rows 0:256 at 2 MACs/cycle) and 1/3 in bf16 — the hybrid
  keeps softmax-input quantization noise at ~1.4e-2 output rel err.

Schedule: per 512-wide k segment, loads/casts/x^T-transposes for segment
N are interleaved between segment N-1's attention chunks (PE never
drains at a boundary); PV matmuls trail their scores by 2 chunks so the
exp latency hides behind the next chunk's score matmuls; a ~3us junk-
matmul warm-up during the initial DMA wait keeps the HAM clock gate
open. Activation transposes run on the TensorEngine (bf16 via identity).
All DRAM traffic is plain HWDGE (hs on the ACT ring, stream on SP);
casts ride DVE/GPSIMD; output leaves as bf16 (cast to f32 on host).

Sharding: 8 cores = batch(4) x LQ-halves(2); no collectives.
"""

import math
from contextlib import ExitStack

import numpy as np

import concourse.bass as bass
import concourse.tile as tile
from concourse import bacc, mybir
from concourse.bass_utils import run_bass_kernel_spmd
from concourse.masks import make_identity

B, LQ_FULL, LK, HID = 4, 1024, 4096, 384
LQ = LQ_FULL // 2  # per-core q shard
P = 128
NHC = HID // P       # 3 hid/feature chunks
SEG = 512            # k segment width
NSEG = LK // SEG     # 8
NKC = LK // P        # 32 k chunks
NQS = LQ // P        # 4 q subtiles
SCALE = 1.0 / math.sqrt(192.0)
BIG = 64.0 / SCALE   # pre-scale additive mask magnitude; exp bias cancels it

F32 = mybir.dt.float32
F8E5 = mybir.dt.float8e5
F8E4 = mybir.dt.float8e4
BF16 = mybir.dt.bfloat16
I32 = mybir.dt.int32

INPUT_NAMES = (
    "hidden_states", "right_hidden_states", "attention_mask",
    "Wq", "bq", "Wk", "bk", "Wv", "bv",
)

_CACHE = {}


def _body(tc, ctx, d, pfx=""):
    nc = tc.nc
    AF = mybir.ActivationFunctionType

    consts = ctx.enter_context(tc.tile_pool(name=f"consts{pfx}", bufs=1))
    stage = ctx.enter_context(tc.tile_pool(name=f"stage{pfx}", bufs=3))
    outp = ctx.enter_context(tc.tile_pool(name=f"outp{pfx}", bufs=2))
    mmps = ctx.enter_context(tc.tile_pool(name=f"mmps{pfx}", bufs=4, space="PSUM"))
    pvps = ctx.enter_context(tc.tile_pool(name=f"pvps{pfx}", bufs=1, space="PSUM"))

    # ---- constants ----
    # ident memset on DVE: the first GPSIMD op has ~1.4us launch latency
    # and the PE warm-up below waits on ident.
    ident = consts.tile([P, P], BF16, name="ident", tag="ident")
    nc.vector.memset(ident, 0.0)
    make_identity(nc, ident, nomemset=True)

    negbig = consts.tile([P, 1], F32, name="negbig", tag="negbig")
    nc.gpsimd.memset(negbig, -64.0)
    # touch Exp once so ACT's table load lands in the idle head, not on the
    # first score tile
    warm = consts.tile([P, 1], F32, name="warm", tag="warm")
    nc.scalar.activation(out=warm, in_=negbig, func=AF.Exp, scale=1.0)

    diag2 = consts.tile([P, 2, 2 * P], F8E5, name="diag2", tag="diag2")
    nc.gpsimd.memset(diag2, 0.0)
    for t in range(2):
        nc.gpsimd.affine_select(
            out=diag2[:, t, t * P:(t + 1) * P], in_=diag2[:, t, t * P:(t + 1) * P],
            compare_op=mybir.AluOpType.not_equal,
            fill=BIG, base=0, pattern=[[-1, P]], channel_multiplier=1,
        )

    # PE warm-up: ~3us of junk matmuls during the initial DMA wait keeps
    # the HAM clock gate open so the prologue runs at full clock. Operand
    # is a DVE-memset scratch so the warm-up starts within ~200ns.
    wtiny = consts.tile([P, 256], BF16, name="wtiny", tag="wtiny")
    nc.vector.memset(wtiny, 0.0)
    wu = mmps.tile([P, 256], F32, name="wu", tag="mm")
    for i in range(16):
        nc.tensor.matmul(wu, lhsT=wtiny[:, 0:P], rhs=wtiny,
                         start=True, stop=True)

    # weights: HWDGE f32 load + DVE cast to bf16 (Wv deferred to epilogue)
    w_sb = {}

    def load_w(wname, on_act=False):
        wf = stage.tile([P, NHC, HID], F32, name="wf", tag="wf", bufs=2)
        nc.sync.dma_start(out=wf, in_=d[wname].rearrange("(c p) h -> p c h", p=P))
        for hc in range(NHC):
            t = consts.tile([P, HID], BF16, name=f"{wname}_{hc}", tag=f"{wname}_{hc}")
            if on_act:
                nc.scalar.activation(out=t, in_=wf[:, hc, :], func=AF.Copy)
            else:
                nc.vector.tensor_copy(out=t, in_=wf[:, hc, :])
            w_sb[wname, hc] = t

    # ---- hs -> hsT (bf16 [hid, q]) via PE transpose ----
    hst = [consts.tile([P, LQ], BF16, name=f"hst{hc}", tag=f"hst{hc}") for hc in range(NHC)]
    hsb = []
    hf = stage.tile([P, NQS, HID], F32, name="hf", tag="hf", bufs=1)
    hs3 = d["hidden_states"].rearrange("(i p) h -> p i h", p=P)
    nc.scalar.dma_start(out=hf[:, 0:2, :], in_=hs3[:, 0:2, :])
    nc.scalar.dma_start(out=hf[:, 2:4, :], in_=hs3[:, 2:4, :])
    for i in range(NQS):
        hb = stage.tile([P, HID], BF16, name="hb", tag="hb", bufs=4)
        nc.vector.tensor_copy(out=hb, in_=hf[:, i, :])
        hsb.append(hb)

    load_w("Wq")
    load_w("Wk")

    bq_sb = consts.tile([P, NHC], F32, name="bq_sb", tag="bq_sb")
    nc.sync.dma_start(out=bq_sb, in_=d["bq"].rearrange("(c p) -> p c", p=P))

    for hc in range(NHC):
        tp = mmps.tile([P, LQ], BF16, name="tp_h", tag="mm")
        for i in range(NQS):
            nc.tensor.transpose(tp[:, i * P:(i + 1) * P], hsb[i][:, hc * P:(hc + 1) * P], ident)
        nc.scalar.activation(out=hst[hc], in_=tp, func=AF.Copy)

    # ---- QT = (hs@Wq + bq)^T  [f, q] ----
    qt = []
    for fc in range(NHC):
        ps_q = mmps.tile([P, LQ], F32, name="ps_q", tag="mm")
        for hc in range(NHC):
            nc.tensor.matmul(
                ps_q, lhsT=w_sb["Wq", hc][:, fc * P:(fc + 1) * P], rhs=hst[hc],
                start=(hc == 0), stop=(hc == NHC - 1),
            )
        t = consts.tile([P, LQ], BF16, name=f"qt{fc}", tag=f"qt{fc}")
        nc.scalar.activation(out=t, in_=ps_q, func=AF.Identity,
                             bias=bq_sb[:, fc:fc + 1], scale=1.0)
        qt.append(t)

    # ---- Wk^T [f, h] via PE transpose, then R = Wk @ Q'^T  [h, q] ----
    # R rows 0:256 land as fp8e4m3 DoubleRow pairs [128, 2, LQ]; rows
    # 256:384 stay bf16 (hybrid: 2/3 of the score contraction in fp8
    # DoubleRow, 1/3 in bf16 keeps softmax input noise within budget).
    wkt = []
    for fc in range(NHC):
        tp = mmps.tile([P, HID], BF16, name="tp_w", tag="mm")
        for hc in range(NHC):
            nc.tensor.transpose(tp[:, hc * P:(hc + 1) * P],
                                w_sb["Wk", hc][:, fc * P:(fc + 1) * P], ident)
        t = consts.tile([P, HID], BF16, name=f"wkt{fc}", tag=f"wkt{fc}")
        nc.scalar.activation(out=t, in_=tp, func=AF.Copy)
        wkt.append(t)
    rp0 = consts.tile([P, 2, LQ], F8E4, name="rp0", tag="rp0")
    r2 = consts.tile([P, LQ], BF16, name="r2", tag="r2")
    for hc in range(NHC):
        ps_r = mmps.tile([P, LQ], F32, name="ps_r", tag="mm")
        for fc in range(NHC):
            nc.tensor.matmul(
                ps_r, lhsT=wkt[fc][:, hc * P:(hc + 1) * P], rhs=qt[fc],
                start=(fc == 0), stop=(fc == NHC - 1),
            )
        if hc < 2:
            nc.scalar.activation(out=rp0[:, hc, :], in_=ps_r, func=AF.Copy)
        else:
            nc.scalar.activation(out=r2, in_=ps_r, func=AF.Copy)

    # ---- per-segment: loads/casts (DMA/DVE/Pool), x^T transposes (PE),
    # attention (PE/ACT). Segment N's transposes are interleaved between
    # segment N-1's attention chunks so the PE never drains at a segment
    # boundary.
    pv_ps = [pvps.tile([P, HID + 1], F32, name=f"pv{qs}", tag=f"pv{qs}") for qs in range(NQS)]
    NSEGS = LK // SEG
    NJ = SEG // P

    def stage_nonpe(seg):
        # Segment 0 splits its loads chunk-fine so the very first attention
        # chunks unblock as early as possible during the pipeline fill.
        fine = seg == 0
        k0 = seg * SEG
        xb4 = []
        xf = stage.tile([P, NJ, HID], F32, name="xf", tag="xf", bufs=3)
        xsrc = (d["right_hidden_states"][k0:k0 + SEG, :]
                .rearrange("(j p) h -> p j h", p=P))
        if fine:
            for j in range(NJ):
                nc.sync.dma_start(out=xf[:, j:j + 1, :], in_=xsrc[:, j:j + 1, :])
        else:
            nc.sync.dma_start(out=xf[:, 0:NJ // 2, :], in_=xsrc[:, 0:NJ // 2, :])
            nc.sync.dma_start(out=xf[:, NJ // 2:NJ, :], in_=xsrc[:, NJ // 2:NJ, :])
        for j in range(NJ):
            xb = stage.tile([P, HID + 1], BF16, name="xb", tag="xb", bufs=16)
            eng = nc.vector if j % 2 == 0 else nc.gpsimd
            eng.tensor_copy(out=xb[:, 0:HID], in_=xf[:, j, :])
            nc.vector.memset(xb[:, HID:HID + 1], 1.0)
            xb4.append(xb)
        # mask chunks: q-subtile pairs -> fp8e5 for DoubleRow. Casts split
        # by k-half and spread Pool/DVE so the first chunks unblock early.
        mbc = []
        mi = stage.tile([P, NQS, SEG], I32, name="mi", tag="mi", bufs=3)
        msrc = (d["attention_mask"][:, k0:k0 + SEG]
                .rearrange("(qs p) k -> p qs k", p=P))
        mts = [stage.tile([P, 2, SEG], F8E5, name="mt", tag="mt", bufs=6)
               for _ in range(NQS // 2)]
        halves = [(pr, h) for h in range(2) for pr in range(NQS // 2)] if fine \
            else [(pr, h) for pr in range(NQS // 2) for h in range(2)]
        done = set()
        for pr, h in halves:
            if fine:
                sl = slice(h * SEG // 2, (h + 1) * SEG // 2)
                nc.sync.dma_start(out=mi[:, 2 * pr:2 * pr + 2, sl],
                                  in_=msrc[:, 2 * pr:2 * pr + 2, sl])
            elif pr not in done:
                done.add(pr)
                nc.sync.dma_start(out=mi[:, 2 * pr:2 * pr + 2, :],
                                  in_=msrc[:, 2 * pr:2 * pr + 2, :])
            sl = slice(h * SEG // 2, (h + 1) * SEG // 2)
            eng = nc.gpsimd if pr == 0 else nc.vector
            eng.tensor_copy(out=mts[pr][:, :, sl], in_=mi[:, 2 * pr:2 * pr + 2, sl])
        mbc = mts
        xp0 = stage.tile([P, 2, SEG], F8E4, name="xp0", tag="xp0", bufs=3)
        xt2 = stage.tile([P, SEG], BF16, name="xt2", tag="xt2", bufs=3)
        return dict(xb4=xb4, mbc=mbc, xp0=xp0, xt2=xt2, k0=k0)

    def stage_pe(sg, hc):
        # x^T transposes for feature chunk hc: features 0:256 land as fp8
        # DoubleRow pairs xp0[:, ko, :] (both halves share one 2-wide PSUM
        # tile -> single DVE copy), features 256:384 bf16.
        if hc == 0:
            tp = mmps.tile([P, 2, SEG], BF16, name="tp_x2", tag="mm")
            sg["tp2"] = tp
            for j in range(NJ):
                nc.tensor.transpose(tp[:, 0, j * P:(j + 1) * P],
                                    sg["xb4"][j][:, 0:P], ident)
        elif hc == 1:
            tp = sg["tp2"]
            for j in range(NJ):
                nc.tensor.transpose(tp[:, 1, j * P:(j + 1) * P],
                                    sg["xb4"][j][:, P:2 * P], ident)
            nc.vector.tensor_copy(out=sg["xp0"], in_=tp)
        else:
            tp = mmps.tile([P, SEG], BF16, name="tp_x", tag="mm")
            for j in range(NJ):
                nc.tensor.transpose(tp[:, j * P:(j + 1) * P],
                                    sg["xb4"][j][:, 2 * P:3 * P], ident)
            nc.scalar.activation(out=sg["xt2"], in_=tp, func=AF.Copy)

    def score(sg, j):
        # S^T + mask accumulation, then exp -> pt. Returns the PV closure,
        # which the caller issues one chunk later so the exp latency hides
        # behind the next chunk's score matmuls.
        kc = sg["k0"] // P + j
        st = mmps.tile([P, LQ], F32, name="st", tag="mm")
        nc.tensor.matmul(
            st, lhsT=sg["xp0"][:, :, j * P:(j + 1) * P], rhs=rp0,
            start=True, stop=False,
            perf_mode=mybir.MatmulPerfMode.DoubleRow,
        )
        nc.tensor.matmul(
            st, lhsT=sg["xt2"][:, j * P:(j + 1) * P], rhs=r2,
            start=False, stop=False,
        )
        for pr in range(NQS // 2):
            nc.tensor.matmul(
                st[:, pr * 2 * P:(pr + 1) * 2 * P],
                lhsT=sg["mbc"][pr][:, :, j * P:(j + 1) * P],
                rhs=diag2, start=False, stop=(pr == NQS // 2 - 1),
                perf_mode=mybir.MatmulPerfMode.DoubleRow,
            )
        pt = stage.tile([P, LQ], BF16, name="pt", tag="pt", bufs=3)
        nc.scalar.activation(out=pt, in_=st, func=AF.Exp, scale=SCALE, bias=negbig)

        def pv():
            for qs in range(NQS):
                nc.tensor.matmul(
                    pv_ps[qs], lhsT=pt[:, qs * P:(qs + 1) * P], rhs=sg["xb4"][j],
                    start=(kc == 0), stop=(kc == NKC - 1),
                )
        return pv

    PV_DEPTH = 2
    pvq = []

    def push_pv(pv):
        pvq.append(pv)
        if len(pvq) > PV_DEPTH:
            pvq.pop(0)()

    prev = stage_nonpe(0)
    for hc in range(NHC):
        stage_pe(prev, hc)
    for seg in range(1, NSEGS):
        cur = stage_nonpe(seg)
        for j in range(NJ):
            if j < NHC:
                stage_pe(cur, j)
            push_pv(score(prev, j))
        prev = cur
    for j in range(NJ):
        push_pv(score(prev, j))
    for pv in pvq:
        pv()

    load_w("Wv", on_act=True)

    bv_d = d["bv"]
    bvf = outp.tile([1, HID], F32, name="bvf", tag="bvf", bufs=1)
    nc.sync.dma_start(
        out=bvf,
        in_=bass.AP(tensor=bv_d.tensor, offset=bv_d.offset, ap=[[0, 1], [1, HID]]),
    )
    bvr = outp.tile([1, HID], BF16, name="bvr", tag="bvr", bufs=1)
    nc.scalar.activation(out=bvr, in_=bvf, func=AF.Copy)
    onesr = outp.tile([1, P], BF16, name="onesr", tag="onesr", bufs=1)
    nc.vector.memset(onesr, 1.0)

    # ---- epilogue: normalize via ACT scale-port (recip early on DVE),
    # transpose, Wv projection + bv via a static ones-row matmul, DMA
    # straight from PSUM. No DVE on the critical path.
    recips = []
    for qs in range(NQS):
        r = stage.tile([P, 1], F32, name="r", tag="r", bufs=4)
        nc.vector.reciprocal(out=r, in_=pv_ps[qs][:, HID:HID + 1])
        recips.append(r)
    pxn = []
    for qs in range(NQS):
        t = outp.tile([P, HID + 1], BF16, name=f"pxn{qs}", tag=f"pxn{qs}", bufs=1)
        if qs % 2 == 0:
            nc.scalar.activation(out=t, in_=pv_ps[qs], func=AF.Copy, scale=recips[qs])
        else:
            nc.vector.tensor_scalar_mul(out=t, in0=pv_ps[qs], scalar1=recips[qs])
        pxn.append(t)
    pxnt = []
    for hc in range(NHC):
        tp = mmps.tile([P, LQ], BF16, name="tp_p", tag="mm")
        for qs in range(NQS):
            nc.tensor.transpose(tp[:, qs * P:(qs + 1) * P], pxn[qs][:, hc * P:(hc + 1) * P], ident)
        t = outp.tile([P, LQ], BF16, name=f"pxnt{hc}", tag=f"pxnt{hc}", bufs=1)
        if hc == 1:
            nc.vector.tensor_copy(out=t, in_=tp)
        else:
            nc.scalar.activation(out=t, in_=tp, func=AF.Copy)
        pxnt.append(t)
    o = outp.tile([P, NQS, HID], BF16, name="o", tag="o", bufs=1)
    for qs in range(NQS):
        ps_o = mmps.tile([P, HID], F32, name="ps_o", tag="mm")
        nc.tensor.matmul(ps_o, lhsT=onesr, rhs=bvr, start=True, stop=False)
        for hc in range(NHC):
            nc.tensor.matmul(
                ps_o, lhsT=pxnt[hc][:, qs * P:(qs + 1) * P], rhs=w_sb["Wv", hc][:, 0:HID],
                start=False, stop=(hc == NHC - 1),
            )
        if qs % 2 == 0:
            nc.vector.tensor_copy(out=o[:, qs, :], in_=ps_o)
        else:
            nc.scalar.activation(out=o[:, qs, :], in_=ps_o, func=AF.Copy)
    odst = d["out"].rearrange("(qs p) h -> p qs h", p=P)
    nc.sync.dma_start(out=odst[:, 0:2, :], in_=o[:, 0:2, :])
    nc.sync.dma_start(out=odst[:, 2:4, :], in_=o[:, 2:4, :])


def _build(repeats=1):
    if ("nc", repeats) in _CACHE:
        return _CACHE["nc", repeats]
    nc = bacc.Bacc(
        "TRN2", target_bir_lowering=False, debug=False,
        enable_asserts=False, num_devices=8,
    )
    d = {
        "hidden_states": nc.dram_tensor("hidden_states", [LQ, HID], F32, kind="ExternalInput").ap(),
        "right_hidden_states": nc.dram_tensor("right_hidden_states", [LK, HID], F32, kind="ExternalInput").ap(),
        "attention_mask": nc.dram_tensor("attention_mask", [LQ, LK], I32, kind="ExternalInput").ap(),
        "Wq": nc.dram_tensor("Wq", [HID, HID], F32, kind="ExternalInput").ap(),
        "bq": nc.dram_tensor("bq", [HID], F32, kind="ExternalInput").ap(),
        "Wk": nc.dram_tensor("Wk", [HID, HID], F32, kind="ExternalInput").ap(),
        "bk": nc.dram_tensor("bk", [HID], F32, kind="ExternalInput").ap(),
        "Wv": nc.dram_tensor("Wv", [HID, HID], F32, kind="ExternalInput").ap(),
        "bv": nc.dram_tensor("bv", [HID], F32, kind="ExternalInput").ap(),
        "out": nc.dram_tensor("out", [LQ, HID], BF16, kind="ExternalOutput").ap(),
    }
    with tile.TileContext(nc) as tc:
        for rep in range(repeats):
            with ExitStack() as ctx:
                _body(tc, ctx, d, pfx=f"_{rep}" if repeats > 1 else "")
    nc.compile()
    _CACHE["nc", repeats] = nc
    return nc


LAST_RESULTS = None


def kernel(hidden_states, right_hidden_states, attention_mask,
           Wq, bq, Wk, bk, Wv, bv):
    global LAST_RESULTS
    import os
    os.environ.setdefault("BASS_NEVER_TRACE", "1")
    nc = _build()
    full = {
        "hidden_states": np.asarray(hidden_states, np.float32),
        "right_hidden_states": np.asarray(right_hidden_states, np.float32),
        "attention_mask": np.asarray(attention_mask, np.int32),
        "Wq": np.asarray(Wq, np.float32), "bq": np.asarray(bq, np.float32),
        "Wk": np.asarray(Wk, np.float32), "bk": np.asarray(bk, np.float32),
        "Wv": np.asarray(Wv, np.float32), "bv": np.asarray(bv, np.float32),
    }
    in_maps = []
    for c in range(8):
        b, h = divmod(c, 2)
        sl = slice(h * LQ, (h + 1) * LQ)
        in_maps.append({
            "hidden_states": np.ascontiguousarray(full["hidden_states"][b, sl]),
            "right_hidden_states": np.ascontiguousarray(full["right_hidden_states"][b]),
            "attention_mask": np.ascontiguousarray(full["attention_mask"][b, sl]),
            "Wq": full["Wq"], "bq": full["bq"],
            "Wk": full["Wk"], "bk": full["bk"],
            "Wv": full["Wv"], "bv": full["bv"],
        })
    res = run_bass_kernel_spmd(nc, in_maps, core_ids=list(range(8)))
    LAST_RESULTS = res
    out = np.empty((B, LQ_FULL, HID), np.float32)
    for c in range(8):
        b, h = divmod(c, 2)
        out[b, h * LQ:(h + 1) * LQ] = res.results[c]["out"].astype(np.float32)
    return out

